# revision 5
# baseline (speedup 1.0000x reference)
"""2-layer GCN on 8 Trainium2 NeuronCores — v3.

v2 + pipelined AllGather: zg is allgathered in 3 panel-range pieces
into 3 separate Shared DRAM tensors (contiguous in/out, one writer
each).  Layer-2 edges are bucketed by the PIECE that owns their source
row (3 gather streams), so stream-p gathers start as soon as piece p
lands — overlapping the collective and early L2 gathers with the tail
of layer-1 compute.
"""
import sys
import numpy as np

sys.path.insert(0, "/opt/trn_rl_repo")

import concourse.bass as bass  # noqa: F401
import concourse.bacc as bacc
import concourse.mybir as mybir
import concourse.tile as tile
from concourse.bass_utils import run_bass_kernel_spmd

P = 128
NCORES = 8
G = 16          # chunks per gather/oh wave
CAP = 7 * P     # max edges per (panel, L1-stream)
NPIECE = 3

F32 = mybir.dt.float32
F16 = mybir.dt.float16
I16 = mybir.dt.int16
AF = mybir.ActivationFunctionType


def _pack_idx(idx_flat):
    n = len(idx_flat)
    n16 = -(-n // 16)
    buf = np.zeros(16 * n16, dtype=np.int16)
    buf[:n] = idx_flat
    blk = buf.reshape(n16, 16).T
    return np.tile(blk, (8, 1)).copy()


def _grid(cnts_max):
    """Universal chunk grid from [npanel, nstream] max edge counts."""
    npanel, ns = cnts_max.shape
    kcnt = -(-cnts_max // P)
    kcnt[:, 0] = np.maximum(1, kcnt[:, 0])   # >=1 chunk per panel
    nch = kcnt.sum(axis=0)
    nw = -(-nch // G)
    seg_off = np.r_[0, np.cumsum(nw * G)][:ns]
    chunks = []
    pos = [0] * ns
    for j in range(npanel):
        nj = int(kcnt[j].sum())
        i = 0
        for st in range(ns):
            for _ in range(int(kcnt[j, st])):
                chunks.append(dict(stream=st, pos=pos[st], panel=j,
                                   col=int(seg_off[st]) + pos[st],
                                   first=(i == 0), last=(i == nj - 1)))
                pos[st] += 1
                i += 1
    return kcnt, nch, seg_off, chunks


def preprocess(edge_index, edge_weight, n):
    row = np.asarray(edge_index[0], dtype=np.int64)
    col = np.asarray(edge_index[1], dtype=np.int64)
    w = np.asarray(edge_weight, dtype=np.float64)
    shard = n // NCORES
    S = -(-n // P)
    R = P * S
    half = (R // 2) & ~15
    assert half < 32768 and R - half < 32768

    deg = np.ones(n, np.float64)
    np.add.at(deg, col, w)
    dinv = 1.0 / np.sqrt(deg)
    wf_edge = (dinv[row] * w * dinv[col]).astype(np.float32)
    wf_self = (dinv * dinv).astype(np.float32)

    core_of = col // shard

    deg_lo = np.zeros(n, np.int64)
    deg_hi = np.zeros(n, np.int64)
    np.add.at(deg_lo, col[row < half], 1)
    np.add.at(deg_hi, col[row >= half], 1)
    selfs_all = np.arange(n)
    deg_lo[selfs_all < half] += 1
    deg_hi[selfs_all >= half] += 1

    blist = []
    for k in range(NCORES):
        lo_c = deg_lo[k * shard:(k + 1) * shard]
        hi_c = deg_hi[k * shard:(k + 1) * shard]
        b = [0]
        cl = ch = cd = 0
        for ld in range(shard):
            if cd == P or cl + lo_c[ld] > CAP or ch + hi_c[ld] > CAP:
                b.append(ld)
                cl = ch = cd = 0
            cl += lo_c[ld]
            ch += hi_c[ld]
            cd += 1
        b.append(shard)
        blist.append(b)
    npanel = max(len(b) - 1 for b in blist)
    bounds = np.zeros((NCORES, npanel + 1), np.int64)
    for k in range(NCORES):
        b = blist[k]
        while len(b) < npanel + 1:
            b.append(shard)
        bounds[k] = b

    # ---- AG pieces over panel ranges
    pb = [0, 2 * npanel // 5, 4 * npanel // 5, npanel]
    prows = [(pb[p + 1] - pb[p]) * P for p in range(NPIECE)]
    piece_of_panel = np.zeros(npanel, np.int64)
    for p in range(NPIECE):
        piece_of_panel[pb[p]:pb[p + 1]] = p
        assert NCORES * prows[p] < 32768, "piece too large for int16 idx"

    # per-node L2 addressing: position inside its piece tensor
    node_panel = np.zeros(n, np.int64)
    node_q = np.zeros(n, np.int64)
    for k in range(NCORES):
        ld = np.arange(shard)
        pj = np.searchsorted(bounds[k], ld, side="right") - 1
        node_panel[k * shard:(k + 1) * shard] = pj
        node_q[k * shard:(k + 1) * shard] = ld - bounds[k][pj]
    node_piece = piece_of_panel[node_panel]
    node_owner = np.arange(n) // shard
    node_pip = np.zeros(n, np.int64)
    for p in range(NPIECE):
        m = node_piece == p
        node_pip[m] = (node_owner[m] * prows[p]
                       + (node_panel[m] - pb[p]) * P + node_q[m])

    # ---- per-core edge lists + universal chunk grids
    cnts1 = np.zeros((NCORES, npanel, 2), np.int64)
    cnts2 = np.zeros((NCORES, npanel, NPIECE), np.int64)
    per_core = []
    for k in range(NCORES):
        m = core_of == k
        selfs = np.arange(k * shard, (k + 1) * shard, dtype=np.int64)
        r_k = np.concatenate([row[m], selfs])
        c_k = np.concatenate([col[m], selfs])
        w_k = np.concatenate([wf_edge[m], wf_self[selfs]])
        ld = c_k - k * shard
        panel = np.searchsorted(bounds[k], ld, side="right") - 1
        q = ld - bounds[k][panel]
        st1 = (r_k >= half).astype(np.int64)
        st2 = node_piece[r_k]
        per_core.append((r_k, w_k, panel, q, st1, st2))
        np.add.at(cnts1[k], (panel, st1), 1)
        np.add.at(cnts2[k], (panel, st2), 1)

    kc1, nch1, soff1, chunks1 = _grid(cnts1.max(axis=0))
    kc2, nch2, soff2, chunks2 = _grid(cnts2.max(axis=0))

    def fill(core_id, nstream, chunks, nch, st, idx_val):
        r_k, w_k, panel, q, st1, st2 = per_core[core_id]
        order = np.lexsort((idx_val, st, panel))
        rv, wv_, pv, qv, sv = (idx_val[order], w_k[order], panel[order],
                               q[order], st[order])
        key = pv * nstream + sv
        cnt = np.bincount(key, minlength=npanel * nstream)
        goff = np.r_[0, np.cumsum(cnt)]
        nw = [-(-int(x) // G) for x in nch]
        idxs = [np.zeros(nw[s] * G * P, np.int64) for s in range(nstream)]
        ncol = sum(x * G for x in nw)
        lc = np.zeros((P, ncol), np.float32)
        wv = np.zeros((P, ncol), np.float32)
        win = np.zeros(npanel * nstream, np.int64)
        for c in chunks:
            j, s = c["panel"], c["stream"]
            g = j * nstream + s
            a = goff[g] + win[g] * P
            b2 = min(goff[g] + win[g] * P + P, goff[g + 1])
            win[g] += 1
            m2 = max(0, b2 - a)
            if m2 > 0:
                pos = c["pos"] * P
                idxs[s][pos:pos + m2] = rv[a:b2]
                lc[:m2, c["col"]] = qv[a:b2]
                wv[:m2, c["col"]] = wv_[a:b2]
        return idxs, lc, wv

    cores = []
    for k in range(NCORES):
        r_k, w_k, panel, q, st1, st2 = per_core[k]
        iv1 = np.where(st1 == 0, r_k, r_k - half)
        idxs1, lc1, wv1 = fill(k, 2, chunks1, nch1, st1, iv1)
        iv2 = node_pip[r_k]
        idxs2, lc2, wv2 = fill(k, NPIECE, chunks2, nch2, st2, iv2)
        cores.append(dict(
            idx_lo=_pack_idx(idxs1[0].astype(np.int16)),
            idx_hi=_pack_idx(idxs1[1].astype(np.int16)),
            idx2_p0=_pack_idx(idxs2[0].astype(np.int16)),
            idx2_p1=_pack_idx(idxs2[1].astype(np.int16)),
            idx2_p2=_pack_idx(idxs2[2].astype(np.int16)),
            raw1=idxs1, raw2=idxs2,
            lc1=lc1, wv1=wv1, lc2=lc2, wv2=wv2,
            bounds=bounds[k].copy()))

    spec = dict(n=n, shard=shard, npanel=npanel, half=half, S=S, R=R,
                pb=pb, prows=prows,
                chunks1=chunks1, chunks2=chunks2,
                nch1=[int(x) for x in nch1], nch2=[int(x) for x in nch2],
                soff1=[int(x) for x in soff1],
                soff2=[int(x) for x in soff2])
    return spec, cores


# ---------------------------------------------------------------- program


def build_program(spec, din, dhid, dout):
    npanel, R, half = spec["npanel"], spec["R"], spec["half"]
    chunks1, chunks2 = spec["chunks1"], spec["chunks2"]
    nch1, nch2 = spec["nch1"], spec["nch2"]
    soff1, soff2 = spec["soff1"], spec["soff2"]
    pb, prows = spec["pb"], spec["prows"]
    nw1 = [-(-x // G) for x in nch1]
    nw2 = [-(-x // G) for x in nch2]
    ncol1 = sum(x * G for x in nw1)
    ncol2 = sum(x * G for x in nw2)
    nrows_pad = npanel * P
    assert din == P and dout == P and dhid == 2 * P

    nc = bacc.Bacc("TRN2", target_bir_lowering=False, debug=False,
                   num_devices=NCORES, num_swdge_queues=4)
    x_d = nc.dram_tensor("x", [R, din], F16, kind="ExternalInput")
    w1_d = nc.dram_tensor("w1", [din, dhid], F16, kind="ExternalInput")
    w2_d = nc.dram_tensor("w2", [dhid, dout], F16, kind="ExternalInput")
    lc1_d = nc.dram_tensor("lc1", [P, ncol1], F16, kind="ExternalInput")
    wv1_d = nc.dram_tensor("wv1", [P, ncol1], F16, kind="ExternalInput")
    lc2_d = nc.dram_tensor("lc2", [P, ncol2], F16, kind="ExternalInput")
    wv2_d = nc.dram_tensor("wv2", [P, ncol2], F16, kind="ExternalInput")
    iot_d = nc.dram_tensor("iot", [P, G * P], F16, kind="ExternalInput")
    ix1_d = [nc.dram_tensor(nm, [P, nw1[s] * G * 8], I16,
                            kind="ExternalInput")
             for s, nm in enumerate(["idx_lo", "idx_hi"])]
    ix2_d = [nc.dram_tensor(nm, [P, nw2[s] * G * 8], I16,
                            kind="ExternalInput")
             for s, nm in enumerate(["idx2_p0", "idx2_p1", "idx2_p2"])]
    out_d = nc.dram_tensor("out", [nrows_pad, dout], F32,
                           kind="ExternalOutput")

    qctr = [0]

    with tile.TileContext(nc) as tc:
        with (
            tc.tile_pool(name="const", bufs=1) as cpool,
            tc.tile_pool(name="dram", bufs=1, space="DRAM") as dram,
        ):
            zg_shard = dram.tile([nrows_pad, dout], F16)
            zgp = [dram.tile([NCORES * prows[p], dout], F16,
                             addr_space="Shared", name=f"zgp{p}",
                             tag=f"zgp{p}")
                   for p in range(NPIECE)]

            def ld(tag, shape, dt_, src):
                t = cpool.tile(shape, dt_, tag=tag)
                nc.sync.dma_start(out=t[:], in_=src)
                return t

            w1_sb = ld("w1", [din, dhid], F16, w1_d[:])
            w2a_sb = ld("w2a", [P, dout], F16, w2_d[0:P, :])
            w2b_sb = ld("w2b", [P, dout], F16, w2_d[P:2 * P, :])
            lc1_sb = ld("lc1", [P, ncol1], F16, lc1_d[:])
            wv1_sb = ld("wv1", [P, ncol1], F16, wv1_d[:])
            lc2_sb = ld("lc2", [P, ncol2], F16, lc2_d[:])
            wv2_sb = ld("wv2", [P, ncol2], F16, wv2_d[:])
            iot_sb = cpool.tile([P, P, G], F16, tag="iot")
            nc.sync.dma_start(out=iot_sb[:],
                              in_=iot_d[:].rearrange("p (d g) -> p d g",
                                                     d=P))
            ix1_sb = [ld(f"ix1_{s}", [P, nw1[s] * G * 8], I16, ix1_d[s][:])
                      for s in range(2)]
            ix2_sb = [ld(f"ix2_{s}", [P, nw2[s] * G * 8], I16, ix2_d[s][:])
                      for s in range(NPIECE)]

            def agg_layer(layer, chunks, streams, lc_sb, wv_sb, soff, nch,
                          gbufs, emit_panel, after_panel=None):
                ns = len(streams)
                import contextlib
                with contextlib.ExitStack() as stk:
                    gp = [stk.enter_context(
                        tc.tile_pool(name=f"g{layer}_{s}", bufs=gbufs))
                        for s in range(ns)]
                    op = [stk.enter_context(
                        tc.tile_pool(name=f"o{layer}_{s}", bufs=gbufs))
                        for s in range(ns)]
                    aggp = stk.enter_context(
                        tc.tile_pool(name=f"agg{layer}", bufs=4,
                                     space="PSUM"))
                    epp = stk.enter_context(
                        tc.tile_pool(name=f"ep{layer}", bufs=2,
                                     space="PSUM"))
                    sbp = stk.enter_context(
                        tc.tile_pool(name=f"sb{layer}", bufs=3))
                    wave_g = [None] * ns
                    wave_oh = [None] * ns
                    psum = None
                    for c in chunks:
                        st, pos, j = c["stream"], c["pos"], c["panel"]
                        wv, slot = divmod(pos, G)
                        if slot == 0:
                            src, idx_sb = streams[st]
                            gsz = min(G, nch[st] - wv * G)
                            t = gp[st].tile([P, G, P], F16, tag="gw")
                            nc.gpsimd.dma_gather(
                                out_ap=t[:, :gsz, :], in_ap=src,
                                idxs_ap=idx_sb[:, wv * G * 8:
                                               wv * G * 8 + gsz * 8],
                                num_idxs=gsz * P, num_idxs_reg=gsz * P,
                                elem_size=P, single_packet=False,
                                queue_num=qctr[0] % 4)
                            qctr[0] += 1
                            oh = op[st].tile([P, P, G], F16, tag="oh")
                            c0 = soff[st] + wv * G
                            nc.vector.tensor_tensor(
                                out=oh[:, :, :gsz], in0=iot_sb[:, :, :gsz],
                                in1=lc_sb[:, None, c0:c0 + gsz]
                                .broadcast_to([P, P, gsz]),
                                op=mybir.AluOpType.is_equal)
                            nc.vector.tensor_tensor(
                                out=oh[:, :, :gsz], in0=oh[:, :, :gsz],
                                in1=wv_sb[:, None, c0:c0 + gsz]
                                .broadcast_to([P, P, gsz]),
                                op=mybir.AluOpType.mult)
                            wave_g[st] = t
                            wave_oh[st] = oh
                        gt = wave_g[st][:, slot, :]
                        ohc = wave_oh[st][:, :, slot]
                        if c["first"]:
                            psum = aggp.tile([P, P], F32, space="PSUM",
                                             tag="agg")
                        nc.tensor.matmul(out=psum[:], lhsT=gt, rhs=ohc,
                                         start=c["first"], stop=c["last"])
                        if c["last"]:
                            emit_panel(j, psum, epp, sbp)
                            if after_panel is not None:
                                after_panel(j)

            # -------- layer 1 ---------------------------------------------
            def l1_panel(j, psum, epp, sbp):
                agg = sbp.tile([P, P], F16, tag="agg_sb")
                nc.scalar.activation(out=agg[:], in_=psum[:], func=AF.Copy)
                h1ps = epp.tile([P, dhid], F32, space="PSUM", tag="h1ps")
                nc.tensor.matmul(out=h1ps[:, 0:P], lhsT=w1_sb[:, 0:P],
                                 rhs=agg[:], start=True, stop=True)
                nc.tensor.matmul(out=h1ps[:, P:2 * P], lhsT=w1_sb[:, P:2 * P],
                                 rhs=agg[:], start=True, stop=True)
                h1 = sbp.tile([P, dhid], F16, tag="h1")
                nc.scalar.activation(out=h1[:], in_=h1ps[:], func=AF.Relu)
                zps = epp.tile([P, dout], F32, space="PSUM", tag="zps")
                nc.tensor.matmul(out=zps[:], lhsT=h1[:, 0:P],
                                 rhs=w2a_sb[:], start=True, stop=False)
                nc.tensor.matmul(out=zps[:], lhsT=h1[:, P:2 * P],
                                 rhs=w2b_sb[:], start=False, stop=True)
                zg = sbp.tile([P, dout], F16, tag="zg")
                nc.scalar.activation(out=zg[:], in_=zps[:], func=AF.Copy)
                nc.sync.dma_start(out=zg_shard[j * P:(j + 1) * P, :],
                                  in_=zg[:])

            def l1_after(j):
                if j + 1 in pb[1:]:
                    p = pb.index(j + 1) - 1
                    a, b = pb[p], pb[p + 1]
                    nc.gpsimd.collective_compute(
                        "AllGather", mybir.AluOpType.bypass,
                        replica_groups=[list(range(NCORES))],
                        ins=[zg_shard[a * P:b * P, :]],
                        outs=[zgp[p].opt()])

            agg_layer(1, chunks1,
                      [(x_d[0:half, :], ix1_sb[0]),
                       (x_d[half:R, :], ix1_sb[1])],
                      lc1_sb, wv1_sb, soff1, nch1, 6, l1_panel,
                      after_panel=l1_after)

            # -------- layer 2 ---------------------------------------------
            def l2_panel(j, psum, epp, sbp):
                o = sbp.tile([P, dout], F32, tag="o2")
                nc.scalar.activation(out=o[:], in_=psum[:], func=AF.Relu)
                nc.sync.dma_start(out=out_d[j * P:(j + 1) * P, :],
                                  in_=o[:])

            agg_layer(2, chunks2,
                      [(zgp[p][:], ix2_sb[p]) for p in range(NPIECE)],
                      lc2_sb, wv2_sb, soff2, nch2, 4, l2_panel)

    nc.compile()
    return nc


# ---------------------------------------------------------------- kernel


def make_inputs(spec, cores, x, W1, W2):
    R = spec["R"]
    n = spec["n"]
    x_pad = np.zeros((R, P), dtype=np.float16)
    x_pad[:n] = x.astype(np.float16)
    W1m = W1.astype(np.float16)
    W2m = W2.astype(np.float16)
    iot = np.broadcast_to(
        np.arange(P, dtype=np.float16)[None, :, None], (P, P, G)
    ).reshape(P, P * G).copy()
    in_maps = []
    for k in range(NCORES):
        c = cores[k]
        in_maps.append(dict(
            x=x_pad, w1=W1m, w2=W2m,
            lc1=c["lc1"].astype(np.float16),
            wv1=c["wv1"].astype(np.float16),
            lc2=c["lc2"].astype(np.float16),
            wv2=c["wv2"].astype(np.float16),
            iot=iot,
            idx_lo=c["idx_lo"], idx_hi=c["idx_hi"],
            idx2_p0=c["idx2_p0"], idx2_p1=c["idx2_p1"],
            idx2_p2=c["idx2_p2"]))
    return in_maps


def unshard(spec, cores, results):
    n, shard, npanel = spec["n"], spec["shard"], spec["npanel"]
    out = np.empty((n, P), dtype=np.float32)
    for k in range(NCORES):
        r = results[k]["out"]
        b = cores[k]["bounds"]
        for j in range(npanel):
            a, e = int(b[j]), int(b[j + 1])
            if e > a:
                out[k * shard + a:k * shard + e] = \
                    r[j * P:(j + 1) * P, :e - a].T
    return out


def kernel(x, edge_index, edge_weight, W1, b1, W2, b2):
    x = np.asarray(x, dtype=np.float32)
    W1 = np.asarray(W1, dtype=np.float32)
    W2 = np.asarray(W2, dtype=np.float32)
    b1 = np.asarray(b1, dtype=np.float32)
    b2 = np.asarray(b2, dtype=np.float32)
    assert not np.any(b1) and not np.any(b2), "bias path not implemented"
    n, din = x.shape
    dhid, dout = W1.shape[1], W2.shape[1]

    spec, cores = preprocess(edge_index, edge_weight, n)
    nc = build_program(spec, din, dhid, dout)
    in_maps = make_inputs(spec, cores, x, W1, W2)
    res = run_bass_kernel_spmd(nc, in_maps, core_ids=list(range(NCORES)))
    return unshard(spec, cores, res.results)
